# revision 1
# baseline (speedup 1.0000x reference)
"""Modulated deformable conv2d (DCNv2) on Trainium2, data-parallel over batch on 8 NeuronCores.

Per-core pipeline (one batch element per core):
  1. prep:  x [256,3136] f32 -> bf16 -> PE-transpose -> xT [3136,256] bf16 in DRAM
            weight [256,2304] f32 -> bf16 -> PE-transpose -> W' tiles [128(k),256(o)]
  2. coords: offsets+mask -> fractional bilinear weights (position-major, per-partition
            scalars) + int16 gather indices (clipped; invalid taps get weight 0)
  3. gather: dma_gather rows of xT for the 4 bilinear neighbors of all 9 taps
  4. combine: w_lt*v_lt + w_rt*v_rt + w_lb*v_lb + w_rb*v_rb  (1 ACT mul + 3 DVE fused ops)
  5. PE-transpose combined tiles into channel-major rhs, 18-K-tile bf16 GEMM, +bias, store
"""

import numpy as np

import concourse.bass as bass
import concourse.bacc as bacc
import concourse.mybir as mybir
import concourse.tile as tile
from concourse.bass_utils import run_bass_kernel_spmd

B, CIN, COUT, H, W = 8, 256, 256, 56, 56
KH = KW = 3
NTAP = 9
HW = H * W  # 3136
NCORES = 8

FP = mybir.dt.float32
BF = mybir.dt.bfloat16
I16 = mybir.dt.int16
AL = mybir.AluOpType
AF = mybir.ActivationFunctionType

# position-dimension splits: (start, valid, nchunks_of_128)
SPLITS = [(0, 1152, 9), (1152, 1152, 9), (2304, 832, 7)]
GT_COLS = sum(2 * NTAP * nch for (_, _, nch) in SPLITS)  # 450


def _gt_colbase(sp):
    return sum(2 * NTAP * SPLITS[i][2] for i in range(sp))


def _emit_prep(nc, tc, x_d, w_d, id_sb, pstp, dramp, wpp):
    """Build xT (DRAM, bf16) and the 18 transposed weight tiles."""
    xT_d = dramp.tile([HW, CIN], BF, tag="xT")
    Wp = []
    with tc.tile_pool(name="prep", bufs=1) as prepp:
        xbs = []
        for cb in range(2):
            xf = prepp.tile([128, HW], FP, tag=f"xf{cb}")
            nc.sync.dma_start(xf[:], x_d.ap()[cb * 128:(cb + 1) * 128, :])
            xb = prepp.tile([128, HW], BF, tag=f"xb{cb}")
            nc.scalar.copy(xb[:], xf[:])
            xbs.append(xb)
        for j in range(25):
            wdt = 128 if j < 24 else 64
            stg = prepp.tile([128, CIN], BF, tag="xstg")
            for cb in range(2):
                ps = pstp.tile([128, 256], BF, tag="ps")
                nc.tensor.transpose(
                    ps[:wdt, :128], xbs[cb][:, j * 128:j * 128 + wdt], id_sb[:]
                )
                nc.vector.tensor_copy(stg[:wdt, cb * 128:(cb + 1) * 128], ps[:wdt, :128])
            nc.sync.dma_start(xT_d[j * 128:j * 128 + wdt, :], stg[:wdt, :])

        wbs = []
        for ob in range(2):
            wf = prepp.tile([128, CIN * NTAP], FP, tag=f"wf{ob}")
            nc.sync.dma_start(wf[:], w_d.ap()[ob * 128:(ob + 1) * 128, :])
            wb = prepp.tile([128, CIN * NTAP], BF, tag=f"wb{ob}")
            nc.scalar.copy(wb[:], wf[:])
            wbs.append(wb)
        for t in range(NTAP):
            for cb in range(2):
                wt = wpp.tile([128, COUT], BF, tag=f"wp{t}_{cb}")
                for ob in range(2):
                    ps = pstp.tile([128, 256], BF, tag="ps")
                    src = wbs[ob][:].rearrange("p (c k) -> p c k", k=NTAP)[
                        :, cb * 128:(cb + 1) * 128, t
                    ]
                    nc.tensor.transpose(ps[:, :128], src, id_sb[:])
                    nc.vector.tensor_copy(wt[:, ob * 128:(ob + 1) * 128], ps[:, :128])
                Wp.append(wt)
    return xT_d, Wp


def _emit_coords(nc, tc, sp, off_d, gt, idf32, coordp, livep, dramp, pstp):
    """Bilinear weights (4x [128, 9*nch] f32) + 16-wrapped int16 gather indices."""
    g0, valid, nch = SPLITS[sp]
    n9 = NTAP * nch
    full_ch = valid // 128
    rem = valid % 128

    # load offsets/mask row-major [27, valid], then PE-transpose each 128-col
    # chunk to the position-major layout offs[p, s*27 + r]
    offn = coordp.tile([32, nch * 128], FP, tag="offn")
    nc.sync.dma_start(offn[0:27, 0:valid], off_d.ap()[:, g0:g0 + valid])
    offs = coordp.tile([128, 27 * nch], FP, tag="offs")
    o3 = offs[:].rearrange("p (s r) -> p r s", r=27)
    if rem:
        nc.vector.memset(offs[rem:128, full_ch * 27:(full_ch + 1) * 27], 0.0)
    for s in range(nch):
        cw = 128 if s < full_ch else rem
        if cw == 0:
            break
        ps = pstp.tile([128, 256], FP, tag="ps")
        nc.tensor.transpose(
            ps[:cw, 0:27], offn[0:27, s * 128:s * 128 + cw], idf32[0:27, 0:27]
        )
        nc.vector.tensor_copy(offs[:cw, s * 27:(s + 1) * 27], ps[:cw, 0:27])
    di = o3[:, 0:18:2, :]
    dj = o3[:, 1:18:2, :]
    mm = o3[:, 18:27, :]
    cb_ = _gt_colbase(sp)
    gtr = gt[:, cb_:cb_ + n9].rearrange("p (t s) -> p t s", s=nch)
    gtc = gt[:, cb_ + n9:cb_ + 2 * n9].rearrange("p (t s) -> p t s", s=nch)

    def T9(tag):
        t_ = coordp.tile([128, n9], FP, tag=tag)
        return t_[:].rearrange("p (t s) -> p t s", s=nch)

    def emit_floor_frac(cc, lo, fr):
        """lo = floor(cc), fr = cc - lo, robust to the f32->int rounding mode."""
        cvi = coordp.tile([128, n9], mybir.dt.int32, tag="cvi")
        nc.vector.tensor_copy(cvi[:].rearrange("p (t s) -> p t s", s=nch), cc)
        cvf = T9("cvf")
        nc.vector.tensor_copy(cvf, cvi[:].rearrange("p (t s) -> p t s", s=nch))
        cmp = T9("cmpf")
        nc.vector.tensor_tensor(cmp, cvf, cc, op=AL.is_gt)
        nc.vector.tensor_sub(lo, cvf, cmp)
        nc.vector.tensor_sub(fr, cc, lo)

    ci = T9("ci")
    nc.vector.tensor_add(ci, di, gtr)
    fi = T9("fi")
    li = T9("li")
    emit_floor_frac(ci, li, fi)
    cj = T9("cj")
    nc.vector.tensor_add(cj, dj, gtc)
    fj = T9("fj")
    lj = T9("lj")
    emit_floor_frac(cj, lj, fj)

    lic = T9("lic")
    nc.vector.tensor_scalar(lic, li, 0.0, 55.0, op0=AL.max, op1=AL.min)
    ljc = T9("ljc")
    nc.vector.tensor_scalar(ljc, lj, 0.0, 55.0, op0=AL.max, op1=AL.min)
    lip = T9("lip")
    nc.vector.tensor_scalar(lip, li, 1.0, None, op0=AL.add)
    ljp = T9("ljp")
    nc.vector.tensor_scalar(ljp, lj, 1.0, None, op0=AL.add)
    ric = T9("ric")
    nc.vector.tensor_scalar(ric, lip, 0.0, 55.0, op0=AL.max, op1=AL.min)
    rjc = T9("rjc")
    nc.vector.tensor_scalar(rjc, ljp, 0.0, 55.0, op0=AL.max, op1=AL.min)

    vi0 = T9("vi0")
    nc.vector.tensor_tensor(vi0, lic, li, op=AL.is_equal)
    vi1 = T9("vi1")
    nc.vector.tensor_tensor(vi1, ric, lip, op=AL.is_equal)
    vj0 = T9("vj0")
    nc.vector.tensor_tensor(vj0, ljc, lj, op=AL.is_equal)
    vj1 = T9("vj1")
    nc.vector.tensor_tensor(vj1, rjc, ljp, op=AL.is_equal)

    a0 = T9("a0")
    nc.vector.tensor_scalar(a0, fi, -1.0, 1.0, op0=AL.mult, op1=AL.add)
    nc.vector.tensor_mul(a0, a0, vi0)
    nc.vector.tensor_mul(a0, a0, mm)
    a1 = T9("a1")
    nc.vector.tensor_mul(a1, fi, vi1)
    nc.vector.tensor_mul(a1, a1, mm)
    b0 = T9("b0")
    nc.vector.tensor_scalar(b0, fj, -1.0, 1.0, op0=AL.mult, op1=AL.add)
    nc.vector.tensor_mul(b0, b0, vj0)
    b1 = T9("b1")
    nc.vector.tensor_mul(b1, fj, vj1)

    wq = []
    for q, (aa, bb) in enumerate(((a0, b0), (a0, b1), (a1, b0), (a1, b1))):
        wt_ = livep.tile([128, n9], FP, tag=f"wq{q}")
        nc.vector.tensor_mul(wt_[:].rearrange("p (t s) -> p t s", s=nch), aa, bb)
        if rem:
            nc.vector.memset(
                wt_[:].rearrange("p (t s) -> p t s", s=nch)[
                    rem:128, :, full_ch:full_ch + 1
                ],
                0.0,
            )
        wq.append(wt_)

    # gather indices, f32 -> int16; idxi col layout = (t*4+q)*nch + s
    idxf = coordp.tile([128, 4 * n9], FP, tag="idxf")
    if4 = idxf[:].rearrange("p (t q s) -> p q t s", q=4, s=nch)
    nc.vector.scalar_tensor_tensor(if4[:, 0], lic, 56.0, ljc, op0=AL.mult, op1=AL.add)
    nc.vector.scalar_tensor_tensor(if4[:, 1], lic, 56.0, rjc, op0=AL.mult, op1=AL.add)
    nc.vector.scalar_tensor_tensor(if4[:, 2], ric, 56.0, ljc, op0=AL.mult, op1=AL.add)
    nc.vector.scalar_tensor_tensor(if4[:, 3], ric, 56.0, rjc, op0=AL.mult, op1=AL.add)
    # int32 row indices in plain [128, (t,q,s)] layout for indirect_dma_start
    idxi = livep.tile([128, 4 * n9], mybir.dt.int32, tag="idxi")
    nc.vector.tensor_copy(idxi[:], idxf[:])
    return wq, idxi


def _emit_split(nc, tc, sp, xT_d, Wp, btiles, id_sb, out_d, wq, idx16,
                gbp, rhsp, cmbp, ostp, pstp, pmmp):
    g0, valid, nch = SPLITS[sp]
    rhs_t = []
    for t in range(NTAP):
        G = gbp.tile([128, 4 * nch * 256], BF, tag="G")
        G3 = G[:].rearrange("p (c e) -> p c e", e=256)
        # verified HW form: dest [128, E] with one row index per partition
        for c in range(4 * nch):
            col = t * 4 * nch + c
            nc.gpsimd.indirect_dma_start(
                G3[:, c, :],
                None,
                xT_d[:],
                bass.IndirectOffsetOnAxis(ap=idx16[:, col:col + 1], axis=0),
            )
        rt = rhsp.tile([128, 2 * nch * 128], BF, tag=f"rhs{t}")
        rt3 = rt[:].rearrange("p (c s e) -> p c s e", c=2, s=nch)
        for s in range(nch):
            tmp0 = cmbp.tile([128, 256], BF, tag="tmp0")
            tmp1 = cmbp.tile([128, 256], BF, tag="tmp1")
            tmp2 = cmbp.tile([128, 256], BF, tag="tmp2")
            vcb = cmbp.tile([128, 256], BF, tag="vcb")

            def wsl(q):
                return wq[q][:, t * nch + s:t * nch + s + 1]

            nc.scalar.activation(tmp0[:], G3[:, 0 * nch + s, :], AF.Copy, scale=wsl(0))
            nc.vector.scalar_tensor_tensor(
                tmp1[:], G3[:, 1 * nch + s, :], wsl(1), tmp0[:],
                op0=AL.mult, op1=AL.add,
            )
            nc.vector.scalar_tensor_tensor(
                tmp2[:], G3[:, 2 * nch + s, :], wsl(2), tmp1[:],
                op0=AL.mult, op1=AL.add,
            )
            nc.vector.scalar_tensor_tensor(
                vcb[:], G3[:, 3 * nch + s, :], wsl(3), tmp2[:],
                op0=AL.mult, op1=AL.add,
            )
            ps = pstp.tile([128, 256], BF, tag="ps")
            for cc in range(2):
                nc.tensor.transpose(
                    ps[:, cc * 128:(cc + 1) * 128],
                    vcb[:, cc * 128:(cc + 1) * 128],
                    id_sb[:],
                )
            nc.vector.tensor_copy(
                rt3[:, :, s, :], ps[:].rearrange("p (c e) -> p c e", c=2)
            )
        rhs_t.append(rt)

    col0 = 0
    while col0 < valid:
        nb = min(512, nch * 128 - col0)
        vb = min(512, valid - col0)
        for ob in range(2):
            pm = pmmp.tile([128, 512], FP, tag="pm")
            kb = 0
            for t in range(NTAP):
                for cc in range(2):
                    nc.tensor.matmul(
                        pm[:, :nb],
                        Wp[t * 2 + cc][:, ob * 128:(ob + 1) * 128],
                        rhs_t[t][:, cc * nch * 128 + col0:cc * nch * 128 + col0 + nb],
                        start=(kb == 0),
                        stop=(kb == 17),
                    )
                    kb += 1
            ot = ostp.tile([128, 512], FP, tag="ot")
            nc.vector.tensor_scalar(
                ot[:, :vb], pm[:, :vb], btiles[ob][:, 0:1], None, op0=AL.add
            )
            nc.sync.dma_start(
                out_d.ap()[ob * 128:(ob + 1) * 128, g0 + col0:g0 + col0 + vb],
                ot[:, :vb],
            )
        col0 += 512


def _emit_kernel(nc, x_d, off_d, w_d, b_d, gt_d, id_d, out_d):
    with tile.TileContext(nc) as tc:
        with (
            tc.tile_pool(name="const", bufs=1) as constp,
            tc.tile_pool(name="wp", bufs=1) as wpp,
            tc.tile_pool(name="pst", bufs=3, space="PSUM") as pstp,
            tc.tile_pool(name="pmm", bufs=4, space="PSUM") as pmmp,
            tc.tile_pool(name="dram", bufs=1, space="DRAM") as dramp,
        ):
            id_sb = constp.tile([128, 128], BF)
            nc.sync.dma_start(id_sb[:], id_d.ap())
            idf32 = constp.tile([128, 128], FP)
            nc.vector.tensor_copy(idf32[:], id_sb[:])
            gt = constp.tile([128, GT_COLS], FP)
            nc.sync.dma_start(gt[:], gt_d.ap())
            btiles = []
            for ob in range(2):
                bt = constp.tile([128, 1], FP, tag=f"bias{ob}")
                nc.sync.dma_start(bt[:], b_d.ap()[ob * 128:(ob + 1) * 128, :])
                btiles.append(bt)

            xT_d, Wp = _emit_prep(nc, tc, x_d, w_d, id_sb, pstp, dramp, wpp)

            with (
                tc.tile_pool(name="gbuf", bufs=2) as gbp,
                tc.tile_pool(name="rhs", bufs=2) as rhsp,
                tc.tile_pool(name="coord", bufs=1) as coordp,
                tc.tile_pool(name="live", bufs=2) as livep,
                tc.tile_pool(name="cmb", bufs=4) as cmbp,
                tc.tile_pool(name="ost", bufs=3) as ostp,
            ):
                for sp in range(len(SPLITS)):
                    wq, idx16 = _emit_coords(nc, tc, sp, off_d, gt, idf32,
                                             coordp, livep, dramp, pstp)
                    _emit_split(nc, tc, sp, xT_d, Wp, btiles, id_sb, out_d,
                                wq, idx16, gbp, rhsp, cmbp, ostp, pstp, pmmp)


def build_nc():
    nc = bacc.Bacc(
        "TRN2",
        target_bir_lowering=False,
        debug=False,
        num_devices=NCORES,
    )
    x_d = nc.dram_tensor("x", [CIN, HW], FP, kind="ExternalInput")
    off_d = nc.dram_tensor("offm", [27, HW], FP, kind="ExternalInput")
    w_d = nc.dram_tensor("w", [COUT, CIN * NTAP], FP, kind="ExternalInput")
    b_d = nc.dram_tensor("bias", [COUT, 1], FP, kind="ExternalInput")
    gt_d = nc.dram_tensor("gtab", [128, GT_COLS], FP, kind="ExternalInput")
    id_d = nc.dram_tensor("ident", [128, 128], BF, kind="ExternalInput")
    out_d = nc.dram_tensor("out", [COUT, HW], FP, kind="ExternalOutput")
    _emit_kernel(nc, x_d, off_d, w_d, b_d, gt_d, id_d, out_d)
    nc.compile()
    return nc


def make_gtab():
    gtab = np.zeros((128, GT_COLS), np.float32)
    p = np.arange(128)
    for sp, (g0, valid, nch) in enumerate(SPLITS):
        cb_ = _gt_colbase(sp)
        for ax in range(2):
            for t in range(NTAP):
                for s in range(nch):
                    g = g0 + s * 128 + p
                    ok = g < g0 + valid
                    gc = np.where(ok, g, 0)
                    if ax == 0:
                        val = gc // 56 + (t // 3 - 1)
                    else:
                        val = gc % 56 + (t % 3 - 1)
                    gtab[:, cb_ + (ax * NTAP + t) * nch + s] = np.where(ok, val, 0.0)
    return gtab


_NC_CACHE = {}


def kernel(x, offset, mask, weight, bias):
    import ml_dtypes

    x = np.ascontiguousarray(np.asarray(x, np.float32))
    offset = np.ascontiguousarray(np.asarray(offset, np.float32))
    mask = np.ascontiguousarray(np.asarray(mask, np.float32))
    weight = np.ascontiguousarray(np.asarray(weight, np.float32))
    bias = np.ascontiguousarray(np.asarray(bias, np.float32))

    if "nc" not in _NC_CACHE:
        _NC_CACHE["nc"] = build_nc()
    nc = _NC_CACHE["nc"]

    gtab = make_gtab()
    ident = np.eye(128, dtype=np.float32).astype(ml_dtypes.bfloat16)
    wmat = np.ascontiguousarray(weight.reshape(COUT, CIN * NTAP))
    bcol = np.ascontiguousarray(bias.reshape(COUT, 1))

    in_maps = []
    for i in range(NCORES):
        offm = np.ascontiguousarray(
            np.concatenate(
                [offset[i].reshape(18, HW), mask[i].reshape(NTAP, HW)], axis=0
            )
        )
        in_maps.append(
            {
                "x": np.ascontiguousarray(x[i].reshape(CIN, HW)),
                "offm": offm,
                "w": wmat,
                "bias": bcol,
                "gtab": gtab,
                "ident": ident,
            }
        )

    res = run_bass_kernel_spmd(nc, in_maps, core_ids=list(range(NCORES)))
    out = np.stack([r["out"] for r in res.results], axis=0)
    return np.ascontiguousarray(out.reshape(B, COUT, H, W).astype(np.float32))



# revision 12
# speedup vs baseline: 2547.0601x; 2547.0601x over previous
"""Modulated deformable conv2d (DCNv2) on Trainium2, data-parallel over batch on 8 NeuronCores.

Per-core pipeline (one batch element per core):
  host prep: xT = x.T as [3136, 256] bf16 (gather source),
             wT = weight pre-transposed to 18 k-tiles [128(k), 256(o)] bf16.
  1. coords: offsets+mask -> bilinear corner weights (position-major, per-partition
             scalars) + int32 gather row indices (clipped; invalid taps weight 0)
  2. gather: batched indirect DMAs (up to 18 indices/partition = 2304
             descriptors per call; SWDGE ring raised to 3072)
  3. combine: custom DUAL_MAC DVE op (in0*s0 + in1*s1) does two corners per
             instruction; ACT takes scale-products on a balanced subset
  4. PE-transpose combined tiles into channel-major rhs, 18-K-tile bf16 GEMM,
     +bias, bf16 store
"""

import numpy as np

import concourse.bass as bass
import concourse.bacc as bacc
import concourse.mybir as mybir
import concourse.tile as tile
import concourse.dve_ops as dve_ops
from concourse.bass_utils import run_bass_kernel_spmd
from concourse.dve_spec import C0, C1, Spec, Src0, Src1, _has_src1, lower
from concourse.dve_uop import DveOpSpec

B, CIN, COUT, H, W = 8, 256, 256, 56, 56
KH = KW = 3
NTAP = 9
HW = H * W  # 3136
NCORES = 8
NKT = 2 * NTAP  # 18 GEMM k-tiles of 128

FP = mybir.dt.float32
BF = mybir.dt.bfloat16
AL = mybir.AluOpType
AF = mybir.ActivationFunctionType

# position-dimension splits: (start, valid, nchunks_of_128)
SPLITS = [(0, 1152, 9), (1152, 1152, 9), (2304, 832, 7)]
GT_COLS = sum(2 * NTAP * nch for (_, _, nch) in SPLITS)  # 450

USE_DUAL = False
GATHER_CHUNK = 1  # HW-verified indirect form: one row index per partition per call


def _register_dual_mac():
    """Register a custom DVE op: out = in0*s0 + in1*s1 (two weighted corners
    per instruction). Follows the documented extension recipe in dve_ops."""
    name = "DUAL_MAC_ANT"
    for op in dve_ops.OPS:
        if op.name == name:
            return op
    spec = Spec(
        body=Src0 * C0 + Src1 * C1,
        reference=lambda in0, in1, s0, s1, imm2: (
            in0.astype(np.float32) * s0 + in1.astype(np.float32) * s1
        ),
    )
    row = dve_ops._CUSTOM_DVE_ROW_BASE + len(dve_ops.OPS)
    assert row < 0x20
    shas = {}
    for ver in ("v3", "v4"):
        try:
            probe = DveOpSpec(
                name=name, opcode=row, uops=lower(spec, ver=ver),
                rd1_en=_has_src1(spec),
            )
            shas[ver] = probe.sha(ver)
        except Exception:
            pass
    op = dve_ops.DveOp(name, spec, subdim=False, uops_sha=shas)
    dve_ops.OPS.append(op)
    dve_ops._SUB_OPCODE_FOR_NAME[name] = row
    dve_ops.CUSTOM_DVE_SPECS[name] = spec
    return op


DUAL_MAC = _register_dual_mac() if USE_DUAL else None


def _gt_colbase(sp):
    return sum(2 * NTAP * SPLITS[i][2] for i in range(sp))


class EngBalance:
    """Greedy static load balancer across DVE and ACT (ns units)."""

    def __init__(self, nc):
        self.nc = nc
        # projected fixed load: DVE coords ~25us; ACT starts free.
        self.cost = {"dve": 25000.0, "act": 0.0}

    def eng(self, name):
        return {"dve": self.nc.vector, "act": self.nc.scalar}[name]

    def pick(self, costs):
        best = min(costs, key=lambda e: self.cost[e] + costs[e])
        self.cost[best] += costs[best]
        return best


def _emit_coords(nc, tc, sp, off_d, gt, idf32, coordp, livep, pstp):
    """Bilinear corner weights (4x [128, 9*nch] f32) + int32 pair-row indices."""
    g0, valid, nch = SPLITS[sp]
    n9 = NTAP * nch
    full_ch = valid // 128
    rem = valid % 128

    offn = coordp.tile([32, nch * 128], FP, tag="offn")
    nc.sync.dma_start(offn[0:27, 0:valid], off_d.ap()[:, g0:g0 + valid])
    offs = coordp.tile([128, 27 * nch], FP, tag="offs")
    o3 = offs[:].rearrange("p (s r) -> p r s", r=27)
    if rem:
        nc.vector.memset(offs[rem:128, full_ch * 27:(full_ch + 1) * 27], 0.0)
    for s in range(nch):
        cw = 128 if s < full_ch else rem
        if cw == 0:
            break
        ps = pstp.tile([128, 256], FP, tag="ps")
        nc.tensor.transpose(
            ps[:cw, 0:27], offn[0:27, s * 128:s * 128 + cw], idf32[0:27, 0:27]
        )
        nc.vector.tensor_copy(offs[:cw, s * 27:(s + 1) * 27], ps[:cw, 0:27])
    di = o3[:, 0:18:2, :]
    dj = o3[:, 1:18:2, :]
    mm = o3[:, 18:27, :]
    cb_ = _gt_colbase(sp)
    gtr = gt[:, cb_:cb_ + n9].rearrange("p (t s) -> p t s", s=nch)
    gtc = gt[:, cb_ + n9:cb_ + 2 * n9].rearrange("p (t s) -> p t s", s=nch)

    def T9(tag):
        t_ = coordp.tile([128, n9], FP, tag=tag)
        return t_[:].rearrange("p (t s) -> p t s", s=nch)

    def emit_floor_frac(cc, lo, fr):
        cvi = coordp.tile([128, n9], mybir.dt.int32, tag="cvi")
        nc.vector.tensor_copy(cvi[:].rearrange("p (t s) -> p t s", s=nch), cc)
        cvf = T9("cvf")
        nc.vector.tensor_copy(cvf, cvi[:].rearrange("p (t s) -> p t s", s=nch))
        cmp = T9("cmpf")
        nc.vector.tensor_tensor(cmp, cvf, cc, op=AL.is_gt)
        nc.vector.tensor_sub(lo, cvf, cmp)
        nc.vector.tensor_sub(fr, cc, lo)

    ci = T9("ci")
    nc.vector.tensor_add(ci, di, gtr)
    fi = T9("fi")
    li = T9("li")
    emit_floor_frac(ci, li, fi)
    cj = T9("cj")
    nc.vector.tensor_add(cj, dj, gtc)
    fj = T9("fj")
    lj = T9("lj")
    emit_floor_frac(cj, lj, fj)

    lic = T9("lic")
    nc.vector.tensor_scalar(lic, li, 0.0, 55.0, op0=AL.max, op1=AL.min)
    ljc = T9("ljc")
    nc.vector.tensor_scalar(ljc, lj, 0.0, 55.0, op0=AL.max, op1=AL.min)
    lip = T9("lip")
    nc.vector.tensor_scalar(lip, li, 1.0, None, op0=AL.add)
    ljp = T9("ljp")
    nc.vector.tensor_scalar(ljp, lj, 1.0, None, op0=AL.add)
    ric = T9("ric")
    nc.vector.tensor_scalar(ric, lip, 0.0, 55.0, op0=AL.max, op1=AL.min)
    rjc = T9("rjc")
    nc.vector.tensor_scalar(rjc, ljp, 0.0, 55.0, op0=AL.max, op1=AL.min)
    vi0 = T9("vi0")
    nc.vector.tensor_tensor(vi0, lic, li, op=AL.is_equal)
    vi1 = T9("vi1")
    nc.vector.tensor_tensor(vi1, ric, lip, op=AL.is_equal)
    vj0 = T9("vj0")
    nc.vector.tensor_tensor(vj0, ljc, lj, op=AL.is_equal)
    vj1 = T9("vj1")
    nc.vector.tensor_tensor(vj1, rjc, ljp, op=AL.is_equal)

    a0 = T9("a0")
    nc.vector.tensor_scalar(a0, fi, -1.0, 1.0, op0=AL.mult, op1=AL.add)
    nc.vector.tensor_mul(a0, a0, vi0)
    nc.vector.tensor_mul(a0, a0, mm)
    a1 = T9("a1")
    nc.vector.tensor_mul(a1, fi, vi1)
    nc.vector.tensor_mul(a1, a1, mm)
    b0 = T9("b0")
    nc.vector.tensor_scalar(b0, fj, -1.0, 1.0, op0=AL.mult, op1=AL.add)
    nc.vector.tensor_mul(b0, b0, vj0)
    b1 = T9("b1")
    nc.vector.tensor_mul(b1, fj, vj1)

    wq = []
    for q, (aa, bb) in enumerate(((a0, b0), (a0, b1), (a1, b0), (a1, b1))):
        wt_ = livep.tile([128, n9], FP, tag=f"wq{q}")
        nc.vector.tensor_mul(wt_[:].rearrange("p (t s) -> p t s", s=nch), aa, bb)
        if rem:
            nc.vector.memset(
                wt_[:].rearrange("p (t s) -> p t s", s=nch)[
                    rem:128, :, full_ch:full_ch + 1
                ],
                0.0,
            )
        wq.append(wt_)

    # gather row indices; col layout = (t*4+q)*nch + s
    idxf = coordp.tile([128, 4 * n9], FP, tag="idxf")
    if4 = idxf[:].rearrange("p (t q s) -> p q t s", q=4, s=nch)
    nc.vector.scalar_tensor_tensor(if4[:, 0], lic, 56.0, ljc, op0=AL.mult, op1=AL.add)
    nc.vector.scalar_tensor_tensor(if4[:, 1], lic, 56.0, rjc, op0=AL.mult, op1=AL.add)
    nc.vector.scalar_tensor_tensor(if4[:, 2], ric, 56.0, ljc, op0=AL.mult, op1=AL.add)
    nc.vector.scalar_tensor_tensor(if4[:, 3], ric, 56.0, rjc, op0=AL.mult, op1=AL.add)
    idxi = livep.tile([128, 4 * n9], mybir.dt.int32, tag="idxi")
    nc.vector.tensor_copy(idxi[:], idxf[:])
    return wq, idxi


def _emit_split(nc, tc, sp, xT_d, wt_all, btiles, id_sb, out_d, wq, idxi, bal,
                gbp, rhsp, cmbp, ostp, pstp, pmmp):
    g0, valid, nch = SPLITS[sp]
    rhs_t = []
    for t in range(NTAP):
        G = gbp.tile([128, 4 * nch * 256], BF, tag="G")
        G3 = G[:].rearrange("p (c e) -> p c e", e=256)
        # HW-verified indirect form: dest [128, 256] with ONE row index per
        # partition per call (multi-column offset APs misbehave on HW)
        ncol = 4 * nch
        for c in range(ncol):
            nc.gpsimd.indirect_dma_start(
                G3[:, c, :],
                None,
                xT_d.ap(),
                bass.IndirectOffsetOnAxis(
                    ap=idxi[:, t * ncol + c:t * ncol + c + 1], axis=0
                ),
            )
        rt = rhsp.tile([128, 2 * nch * 128], BF, tag=f"rhs{t}")
        rt3 = rt[:].rearrange("p (c s e) -> p c s e", c=2, s=nch)
        for s in range(nch):
            lt = G3[:, 0 * nch + s, :]
            rtc = G3[:, 1 * nch + s, :]
            lb = G3[:, 2 * nch + s, :]
            rb = G3[:, 3 * nch + s, :]

            def wsl(q):
                return wq[q][:, t * nch + s:t * nch + s + 1]

            vcb = cmbp.tile([128, 256], BF, tag="vcb")
            utop = cmbp.tile([128, 256], BF, tag="utop")
            ubot = cmbp.tile([128, 256], BF, tag="ubot")
            if DUAL_MAC is not None:
                # A-form: both DUALs on DVE.  B-form: top pair as two ACT
                # products + a DVE add.  Greedy choice balances DVE vs ACT.
                pa = bal.cost["dve"] + 847.0
                pb = max(bal.cost["dve"] + 713.0, bal.cost["act"] + 796.0)
                if pa <= pb:
                    bal.cost["dve"] += 847.0
                    nc.vector._custom_dve(
                        DUAL_MAC, out=utop[:], in0=lt, in1=rtc,
                        s0=wsl(0), s1=wsl(1),
                    )
                    nc.vector._custom_dve(
                        DUAL_MAC, out=ubot[:], in0=lb, in1=rb,
                        s0=wsl(2), s1=wsl(3),
                    )
                else:
                    bal.cost["dve"] += 713.0
                    bal.cost["act"] += 796.0
                    p0 = cmbp.tile([128, 256], BF, tag="p0")
                    nc.scalar.activation(p0[:], lt, AF.Copy, scale=wsl(0))
                    p1 = cmbp.tile([128, 256], BF, tag="p1")
                    nc.scalar.activation(p1[:], rtc, AF.Copy, scale=wsl(1))
                    nc.vector.tensor_add(utop[:], p0[:], p1[:])
                    nc.vector._custom_dve(
                        DUAL_MAC, out=ubot[:], in0=lb, in1=rb,
                        s0=wsl(2), s1=wsl(3),
                    )
                nc.vector.tensor_add(vcb[:], utop[:], ubot[:])
            else:
                # fallback: ACT product + 3 fused DVE mul-adds
                nc.scalar.activation(utop[:], lt, AF.Copy, scale=wsl(0))
                nc.vector.scalar_tensor_tensor(
                    ubot[:], rtc, wsl(1), utop[:], op0=AL.mult, op1=AL.add)
                nc.vector.scalar_tensor_tensor(
                    utop[:], lb, wsl(2), ubot[:], op0=AL.mult, op1=AL.add)
                nc.vector.scalar_tensor_tensor(
                    vcb[:], rb, wsl(3), utop[:], op0=AL.mult, op1=AL.add)
                bal.cost["act"] += 398.0
                bal.cost["dve"] += 981.0
            ps = pstp.tile([128, 256], BF, tag="ps")
            for cc in range(2):
                nc.tensor.transpose(
                    ps[:, cc * 128:(cc + 1) * 128],
                    vcb[:, cc * 128:(cc + 1) * 128],
                    id_sb[:],
                )
            e = bal.pick({"dve": 258.0, "act": 398.0})
            if e == "act":
                nc.scalar.copy(
                    rt3[:, :, s, :], ps[:].rearrange("p (c e) -> p c e", c=2)
                )
            else:
                nc.vector.tensor_copy(
                    rt3[:, :, s, :], ps[:].rearrange("p (c e) -> p c e", c=2)
                )
        rhs_t.append(rt)

    col0 = 0
    while col0 < valid:
        nb = min(512, nch * 128 - col0)
        vb = min(512, valid - col0)
        for ob in range(2):
            pm = pmmp.tile([128, 512], FP, tag="pm")
            kb = 0
            for t in range(NTAP):
                for cc in range(2):
                    k = t * 2 + cc
                    nc.tensor.matmul(
                        pm[:, :nb],
                        wt_all[:, k * 256 + ob * 128:k * 256 + (ob + 1) * 128],
                        rhs_t[t][:, cc * nch * 128 + col0:cc * nch * 128 + col0 + nb],
                        start=(kb == 0),
                        stop=(kb == NKT - 1),
                    )
                    kb += 1
            ot = ostp.tile([128, 512], BF, tag="ot")
            e = bal.pick({"dve": 650.0, "act": 570.0})
            if e == "act":
                nc.scalar.activation(
                    ot[:, :vb], pm[:, :vb], AF.Identity, bias=btiles[ob][:, 0:1]
                )
            else:
                nc.vector.tensor_scalar(
                    ot[:, :vb], pm[:, :vb], btiles[ob][:, 0:1], None, op0=AL.add
                )
            nc.sync.dma_start(
                out_d.ap()[ob * 128:(ob + 1) * 128, g0 + col0:g0 + col0 + vb],
                ot[:, :vb],
            )
        col0 += 512


def _emit_kernel(nc, xT_d, off_d, wT_d, b_d, gt_d, id_d, out_d):
    with tile.TileContext(nc) as tc:
        with (
            tc.tile_pool(name="const", bufs=1) as constp,
            tc.tile_pool(name="pst", bufs=3, space="PSUM") as pstp,
            tc.tile_pool(name="pmm", bufs=4, space="PSUM") as pmmp,
        ):
            id_sb = constp.tile([128, 128], BF)
            nc.sync.dma_start(id_sb[:], id_d.ap())
            idf32 = constp.tile([128, 128], FP)
            nc.vector.tensor_copy(idf32[:], id_sb[:])
            gt = constp.tile([128, GT_COLS], FP)
            nc.sync.dma_start(gt[:], gt_d.ap())
            btiles = []
            for ob in range(2):
                bt = constp.tile([128, 1], FP, tag=f"bias{ob}")
                nc.sync.dma_start(bt[:], b_d.ap()[ob * 128:(ob + 1) * 128, :])
                btiles.append(bt)
            wt_all = constp.tile([128, NKT * 256], BF)
            nc.sync.dma_start(
                wt_all[:].rearrange("p (k o) -> p k o", k=NKT),
                wT_d.ap().rearrange("(k p) o -> p k o", k=NKT),
            )

            bal = EngBalance(nc)
            with (
                tc.tile_pool(name="gbuf", bufs=2) as gbp,
                tc.tile_pool(name="rhs", bufs=2) as rhsp,
                tc.tile_pool(name="coord", bufs=1) as coordp,
                tc.tile_pool(name="live", bufs=2) as livep,
                tc.tile_pool(name="cmb", bufs=4) as cmbp,
                tc.tile_pool(name="ost", bufs=3) as ostp,
            ):
                for sp in range(len(SPLITS)):
                    wq, idxi = _emit_coords(nc, tc, sp, off_d, gt, idf32,
                                            coordp, livep, pstp)
                    _emit_split(nc, tc, sp, xT_d, wt_all, btiles, id_sb, out_d,
                                wq, idxi, bal, gbp, rhsp, cmbp, ostp, pstp, pmmp)


def build_nc():
    nc = bacc.Bacc(
        "TRN2",
        target_bir_lowering=False,
        debug=False,
        num_devices=NCORES,
    )
    xT_d = nc.dram_tensor("xT", [HW, CIN], BF, kind="ExternalInput")
    off_d = nc.dram_tensor("offm", [27, HW], FP, kind="ExternalInput")
    wT_d = nc.dram_tensor("wT", [NKT * 128, COUT], BF, kind="ExternalInput")
    b_d = nc.dram_tensor("bias", [COUT, 1], FP, kind="ExternalInput")
    gt_d = nc.dram_tensor("gtab", [128, GT_COLS], FP, kind="ExternalInput")
    id_d = nc.dram_tensor("ident", [128, 128], BF, kind="ExternalInput")
    out_d = nc.dram_tensor("out", [COUT, HW], BF, kind="ExternalOutput")
    _emit_kernel(nc, xT_d, off_d, wT_d, b_d, gt_d, id_d, out_d)
    nc.compile()
    return nc


def make_gtab():
    gtab = np.zeros((128, GT_COLS), np.float32)
    p = np.arange(128)
    for sp, (g0, valid, nch) in enumerate(SPLITS):
        cb_ = _gt_colbase(sp)
        for ax in range(2):
            for t in range(NTAP):
                for s in range(nch):
                    g = g0 + s * 128 + p
                    ok = g < g0 + valid
                    gc = np.where(ok, g, 0)
                    if ax == 0:
                        val = gc // 56 + (t // 3 - 1)
                    else:
                        val = gc % 56 + (t % 3 - 1)
                    gtab[:, cb_ + (ax * NTAP + t) * nch + s] = np.where(ok, val, 0.0)
    return gtab


def make_in_maps(x, offset, mask, weight, bias):
    import ml_dtypes

    x = np.ascontiguousarray(np.asarray(x, np.float32))
    offset = np.ascontiguousarray(np.asarray(offset, np.float32))
    mask = np.ascontiguousarray(np.asarray(mask, np.float32))
    weight = np.ascontiguousarray(np.asarray(weight, np.float32))
    bias = np.ascontiguousarray(np.asarray(bias, np.float32))

    gtab = make_gtab()
    ident = np.eye(128, dtype=np.float32).astype(ml_dtypes.bfloat16)
    # wT[(t*2+cb)*128 + i, o] = weight[o, cb*128+i, t]
    w9 = weight.reshape(COUT, CIN, NTAP)
    wT = np.ascontiguousarray(
        np.transpose(w9, (2, 1, 0)).reshape(NKT * 128, COUT)
    ).astype(ml_dtypes.bfloat16)
    bcol = np.ascontiguousarray(bias.reshape(COUT, 1))

    in_maps = []
    for i in range(NCORES):
        offm = np.ascontiguousarray(
            np.concatenate(
                [offset[i].reshape(18, HW), mask[i].reshape(NTAP, HW)], axis=0
            )
        )
        xTp = np.ascontiguousarray(
            x[i].reshape(CIN, HW).T.astype(ml_dtypes.bfloat16)
        )
        in_maps.append(
            {
                "xT": xTp,
                "offm": offm,
                "wT": wT,
                "bias": bcol,
                "gtab": gtab,
                "ident": ident,
            }
        )
    return in_maps


_NC_CACHE = {}


def kernel(x, offset, mask, weight, bias):
    if "nc" not in _NC_CACHE:
        _NC_CACHE["nc"] = build_nc()
    nc = _NC_CACHE["nc"]

    in_maps = make_in_maps(x, offset, mask, weight, bias)
    res = run_bass_kernel_spmd(nc, in_maps, core_ids=list(range(NCORES)))
    out = np.stack([r["out"] for r in res.results], axis=0)
    return np.ascontiguousarray(
        out.reshape(B, COUT, H, W).astype(np.float32)
    )
